# revision 9
# baseline (speedup 1.0000x reference)
"""Trainium2 Bass kernel for nn_DiscontinuedGRU (bidirectional masked GRU).

Math (per direction, torch GRUCell):
    gx = x @ Wih.T + bih ; gh = h @ Whh.T + bhh
    r = sigmoid(gxr+ghr); z = sigmoid(gxz+ghz); n = tanh(gxn + r*ghn)
    h' = (1-z)*n + z*h_in,  h_in = 0 whenever the mask resets (else prev h)

Key idea: the Bernoulli(0.5) reset mask D chops every (batch, direction)
sequence into independent segments (mean length 2, max ~20).  Host code
derives the segments from D, sorts them by length, and the device processes
"round k" = the k-th element of every segment as one large batched GRU-cell
evaluation (big matmuls + gates).  Sequential depth collapses from 2048
steps to ~20 rounds.  Batch is sharded 8 ways (8 batch/core); each core
runs both directions for its slice.

Device layout: feature-on-partition (H=256 -> 2 chunks of 128).  x rows are
gathered+transposed by dma_gather(transpose=True) directly into matmul rhs
layout.  Round outputs are dumped densely (feature-major, round-compacted
column order) and the host inverse-permutes into the final (SEQ,B,2H)
output.
"""

import os
import sys
from contextlib import ExitStack

for _p in ("/opt/trn_rl_repo", "/root/.axon_site/_ro/trn_rl_repo"):
    if os.path.isdir(_p) and _p not in sys.path:
        sys.path.insert(0, _p)

import numpy as np
import ml_dtypes

ABL = set(os.environ.get("GRU_ABLATE", "").split(","))

import concourse.bass as bass
import concourse.tile as tile
from concourse import bacc, mybir
from concourse.bass_utils import run_bass_kernel_spmd

BF = ml_dtypes.bfloat16
F32 = mybir.dt.float32
BF16 = mybir.dt.bfloat16
I16 = mybir.dt.int16

SEQ, B, I, H = 2048, 64, 256, 256
NCORES = 8
CHUNK = 512

Sigmoid = mybir.ActivationFunctionType.Sigmoid
Tanh = mybir.ActivationFunctionType.Tanh
MULT = mybir.AluOpType.mult
ADD = mybir.AluOpType.add
SUB = mybir.AluOpType.subtract


# ----------------------------------------------------------------- planning

def _segments(Dloc, reverse):
    """Segments of one direction for one core's (T, BL) mask slice.

    Returns list of (length, start_t, b) sorted by length descending.
    Forward: reset at t if t==0 or D[t]==1; segment runs forward from t.
    Backward: reset at t if t==T-1 or D[t+1]==1; segment runs backward.
    """
    T, BL = Dloc.shape
    segs = []
    for b in range(BL):
        if not reverse:
            m = Dloc[:, b] == 1
            m = m.copy()
            m[0] = True
            starts = np.flatnonzero(m)
            lens = np.diff(np.append(starts, T))
        else:
            m = np.zeros(T, dtype=bool)
            m[:-1] = Dloc[1:, b] == 1
            m[T - 1] = True
            starts = np.flatnonzero(m)
            lens = np.diff(np.concatenate([[-1], starts]))
        for s, L in zip(starts.tolist(), lens.tolist()):
            segs.append((int(L), int(s), int(b)))
    segs.sort(key=lambda x: -x[0])
    return segs


def _round_up(v, m):
    return (v + m - 1) // m * m


class Plan:
    pass


def make_plan(D, T=SEQ, ncores=NCORES):
    """Global round structure + per-core gather indices & output col maps."""
    BL = D.shape[1] // ncores
    p = Plan()
    p.T, p.BL, p.ncores = T, BL, ncores
    p.core_segs = []  # [core][dir] -> sorted seg list
    for c in range(ncores):
        Dloc = np.asarray(D[:, c * BL:(c + 1) * BL])
        p.core_segs.append([_segments(Dloc, False), _segments(Dloc, True)])

    p.K = [0, 0]          # rounds per dir
    p.Nk = [[], []]       # padded global round sizes per dir
    p.offs = [[], []]     # col offset of each round in the dump tensor
    p.PT = [0, 0]
    for d in range(2):
        K = max(segs[d][0][0] for segs in p.core_segs)
        p.K[d] = K
        off = 0
        for k in range(K):
            n_glob = max(sum(1 for L, _, _ in segs[d] if L > k)
                         for segs in p.core_segs)
            nk = _round_up(max(n_glob, 128), 128)
            if k > 0:
                nk = min(nk, p.Nk[d][k - 1])  # never exceed written prefix
            p.Nk[d].append(nk)
            p.offs[d].append(off)
            off += nk
        p.PT[d] = off
    p.NB1 = [_round_up(p.Nk[d][1], CHUNK) if p.K[d] > 1 else CHUNK
             for d in range(2)]

    # per-core gather index arrays (wrapped 16-partition layout) and
    # col -> output-row maps for host-side assembly
    p.gidx = []     # [core] -> (128, GCOLS) int16
    p.col2row = []  # [core][dir] -> (PT,) int32, -1 = padding
    gcols = sum(p.PT) // 16
    p.GCOLS = gcols
    p.gc0 = [0, p.PT[0] // 16]
    for c in range(ncores):
        gi = np.zeros((16, gcols), dtype=np.int16)
        maps = []
        for d in range(2):
            segs = p.core_segs[c][d]
            c2r = np.full(p.PT[d], -1, dtype=np.int32)
            for k in range(p.K[d]):
                nk = p.Nk[d][k]
                nreal = sum(1 for L, _, _ in segs if L > k)
                rows = np.zeros(nk, dtype=np.int64)
                for j in range(nreal):
                    L, s, b = segs[j]
                    t = s + k if d == 0 else s - k
                    rows[j] = t * BL + b
                    c2r[p.offs[d][k] + j] = t * BL + b
                blk = rows.reshape(nk // 16, 16).T.astype(np.int16)
                a0 = p.gc0[d] + p.offs[d][k] // 16
                gi[:, a0:a0 + nk // 16] = blk
            maps.append(c2r)
        p.gidx.append(np.tile(gi, (8, 1)))
        p.col2row.append(maps)
    return p


# ----------------------------------------------------------------- builder

def build_program(p):
    """Emit the SPMD Bass/Tile program for plan p."""
    T, BL = p.T, p.BL
    ROWS = T * BL
    nc = bacc.Bacc("TRN2", target_bir_lowering=False, debug=False,
                   num_devices=p.ncores)

    X_d = nc.dram_tensor("xb", [ROWS, I], BF16, kind="ExternalInput")
    gidx_d = nc.dram_tensor("gidx", [128, p.GCOLS], I16, kind="ExternalInput")
    w_d = [[nc.dram_tensor(f"w{nm}{d}", [2, 128, 3 * H], BF16,
                           kind="ExternalInput")
            for nm in ("ih", "hh")] for d in range(2)]
    bias_d = nc.dram_tensor("biasbf", [1, 2048], BF16, kind="ExternalInput")
    bnhh_d = nc.dram_tensor("bnhh32", [128, 4], F32, kind="ExternalInput")
    ident_d = nc.dram_tensor("identbf", [128, 128], BF16, kind="ExternalInput")
    ones_d = nc.dram_tensor("onesbf", [1, CHUNK], BF16, kind="ExternalInput")
    hout_dt = BF16 if "syncdump" in ABL else F32
    hout_d = [nc.dram_tensor(f"h{'fb'[d]}", [128, 2, p.PT[d]], hout_dt,
                             kind="ExternalOutput") for d in range(2)]

    with tile.TileContext(nc) as tc, ExitStack() as ctx:
        cpool = ctx.enter_context(tc.tile_pool(name="consts", bufs=1))
        wt = [[cpool.tile([128, 2, 3 * H], BF16, tag=f"w{i_m}{d}", name=f"w{i_m}{d}")
               for i_m in range(2)] for d in range(2)]
        for d in range(2):
            for i_m in range(2):
                for kk in range(2):
                    nc.sync.dma_start(wt[d][i_m][:, kk, :],
                                      w_d[d][i_m].ap()[kk])
        bias_t = cpool.tile([1, 2048], BF16, tag="biasbf")
        nc.sync.dma_start(bias_t[:], bias_d.ap())
        bnhh_t = cpool.tile([128, 4], F32, tag="bnhh32")
        nc.sync.dma_start(bnhh_t[:], bnhh_d.ap())
        ident_t = cpool.tile([128, 128], BF16, tag="identbf")
        nc.sync.dma_start(ident_t[:], ident_d.ap())
        ones_t = cpool.tile([1, CHUNK], BF16, tag="onesbf")
        nc.sync.dma_start(ones_t[:], ones_d.ap())
        gidx_t = cpool.tile([128, p.GCOLS], I16, tag="gidx")
        nc.sync.dma_start(gidx_t[:], gidx_d.ap())

        hpool = [ctx.enter_context(tc.tile_pool(name=f"hbuf{d}", bufs=2))
                 for d in range(2)]
        xpool = ctx.enter_context(tc.tile_pool(name="xg", bufs=3))
        rzpool = ctx.enter_context(tc.tile_pool(name="rz", bufs=2))
        t1pool = ctx.enter_context(tc.tile_pool(name="t1", bufs=2))
        npool = ctx.enter_context(tc.tile_pool(name="nt", bufs=2))
        dpool = ctx.enter_context(tc.tile_pool(name="dt", bufs=2))
        epool = ctx.enter_context(tc.tile_pool(name="et", bufs=2))
        tpool = ctx.enter_context(tc.tile_pool(name="htr", bufs=3))
        prz = ctx.enter_context(tc.tile_pool(name="prz", bufs=1, space="PSUM"))
        pnh = ctx.enter_context(tc.tile_pool(name="pnh", bufs=1, space="PSUM"))
        pni = ctx.enter_context(tc.tile_pool(name="pni", bufs=1, space="PSUM"))

        state = {"hprev": [None, None], "hnew": [None, None]}

        def emit_chunk(d, k, j0, n):
            hprev = state["hprev"]
            hnew = state["hnew"]
            bb = 1024 * d
            xt = xpool.tile([128, 2, n], BF16, tag="xg", name="xt")
            if "dxbar" in ABL:
                for cc in range(2):
                    nc.sync.dma_start(xt[:, cc, :],
                                      X_d.ap()[0:n, cc * 128:(cc + 1) * 128],
                                      transpose=True)
            elif "nogather" not in ABL:
                nc.gpsimd.dma_gather(
                    out_ap=xt[:],
                    in_ap=X_d.ap(),
                    idxs_ap=gidx_t[:, p.gc0[d] + (p.offs[d][k] + j0) // 16:
                                   p.gc0[d] + (p.offs[d][k] + j0 + n) // 16],
                    num_idxs=n, num_idxs_reg=n, elem_size=I, transpose=True,
                )
            # PSUM tiles are allocated full-width so each m-slice owns a
            # whole bank (avoids interleaved accumulation groups in a bank).
            if "nomm" in ABL:
                c0 = p.offs[d][k] + j0
                nc.sync.dma_start(hout_d[d].ap()[:, :, c0:c0 + n], xt[:])
                return
            P_rz = prz.tile([128, 4, CHUNK], F32, tag="prz", name="P_rz")
            for m in range(4):
                nc.tensor.matmul(P_rz[:, m, 0:n],
                                 bias_t[0:1, bb + m * 128:bb + m * 128 + 128],
                                 ones_t[0:1, 0:n], start=True, stop=False)
                for kk in range(2):
                    nc.tensor.matmul(P_rz[:, m, 0:n],
                                     wt[d][0][:, kk, m * 128:m * 128 + 128],
                                     xt[:, kk, :],
                                     start=False, stop=(k == 0 and kk == 1))
                if k > 0:
                    for kk in range(2):
                        nc.tensor.matmul(P_rz[:, m, 0:n],
                                         wt[d][1][:, kk, m * 128:m * 128 + 128],
                                         hprev[d][:, kk, j0:j0 + n],
                                         start=False, stop=(kk == 1))
            P_ni = pni.tile([128, 2, CHUNK], F32, tag="pni", name="P_ni")
            for mm in range(2):
                m = 4 + mm
                nc.tensor.matmul(P_ni[:, mm, 0:n],
                                 bias_t[0:1, bb + 512 + mm * 128:
                                        bb + 512 + mm * 128 + 128],
                                 ones_t[0:1, 0:n], start=True, stop=False)
                for kk in range(2):
                    nc.tensor.matmul(P_ni[:, mm, 0:n],
                                     wt[d][0][:, kk, m * 128:m * 128 + 128],
                                     xt[:, kk, :], start=False, stop=False)
            if k > 0:
                P_nh = pnh.tile([128, 2, CHUNK], F32, tag="pnh", name="P_nh")
                for mm in range(2):
                    m = 4 + mm
                    nc.tensor.matmul(P_nh[:, mm, 0:n],
                                     bias_t[0:1, bb + 768 + mm * 128:
                                            bb + 768 + mm * 128 + 128],
                                     ones_t[0:1, 0:n], start=True, stop=False)
                    for kk in range(2):
                        nc.tensor.matmul(P_nh[:, mm, 0:n],
                                         wt[d][1][:, kk, m * 128:m * 128 + 128],
                                         hprev[d][:, kk, j0:j0 + n],
                                         start=False, stop=(kk == 1))
            rz = rzpool.tile([128, 4, n], BF16, tag="rz", name="rz")
            nc.scalar.activation(rz[:], P_rz[:, :, 0:n], Sigmoid)
            if "nogates" in ABL:
                c0 = p.offs[d][k] + j0
                if "nodump" not in ABL:
                    nc.sync.dma_start(hout_d[d].ap()[:, :, c0:c0 + n],
                                      rz[:, 0:2, :])
                return
            t1 = t1pool.tile([128, 2, n], BF16, tag="t1", name="t1")
            if k > 0:
                nc.vector.tensor_tensor(t1[:], rz[:, 0:2, :],
                                        P_nh[:, :, 0:n], MULT)
            else:
                for mm in range(2):
                    nc.vector.tensor_scalar_mul(
                        t1[:, mm, :], rz[:, mm, :],
                        bnhh_t[:, 2 * d + mm:2 * d + mm + 1])
            for mm in range(2):
                nc.tensor.matmul(P_ni[:, mm, 0:n], ident_t[:],
                                 t1[:, mm, :], start=False, stop=True)
            nt = npool.tile([128, 2, n], BF16, tag="nt", name="nt")
            nc.scalar.activation(nt[:], P_ni[:, :, 0:n], Tanh)

            in_buf = (k + 1 < p.K[d]) and (j0 + n <= p.NB1[d])
            hd = hnew[d][:, :, j0:j0 + n] if in_buf \
                else tpool.tile([128, 2, n], BF16, tag="htr", name="htr")[:]
            et = epool.tile([128, 2, n], BF16, tag="et", name="et")
            if k > 0:
                dt = dpool.tile([128, 2, n], BF16, tag="dt", name="dt")
                nc.vector.tensor_tensor(dt[:], hprev[d][:, :, j0:j0 + n],
                                        nt[:], SUB)
                nc.vector.tensor_tensor(et[:], rz[:, 2:4, :], dt[:], MULT)
                nc.vector.tensor_tensor(hd, nt[:], et[:], ADD)
            else:
                nc.vector.tensor_tensor(et[:], rz[:, 2:4, :], nt[:], MULT)
                nc.vector.tensor_tensor(hd, nt[:], et[:], SUB)
            c0 = p.offs[d][k] + j0
            if "syncdump" in ABL:
                nc.sync.dma_start(hout_d[d].ap()[:, :, c0:c0 + n], hd)
            elif "nodump" not in ABL:
                nc.gpsimd.dma_start(hout_d[d].ap()[:, :, c0:c0 + n], hd)

        maxK = max(p.K)
        for _rep in range(int(os.environ.get("GRU_REPEAT", "1"))):
            hprev = state["hprev"] = [None, None]
            hnew = state["hnew"] = [None, None]
            for k in range(maxK):
                for d in range(2):
                    if k >= p.K[d]:
                        continue
                    if k + 1 < p.K[d]:
                        hnew[d] = hpool[d].tile([128, 2, p.NB1[d]], BF16,
                                                tag=f"hbuf{d}",
                                                name=f"hbuf{d}")
                    nk = p.Nk[d][k]
                    for j0 in range(0, nk, CHUNK):
                        emit_chunk(d, k, j0, min(CHUNK, nk - j0))
                    hprev[d], hnew[d] = hnew[d], None

    nc.compile()
    return nc


# ------------------------------------------------------------- host driver

def _shared_consts(Wih_f, Whh_f, bih_f, bhh_f, Wih_b, Whh_b, bih_b, bhh_b):
    wb = {}
    for d, (Wih, Whh, bih, bhh) in enumerate(
            [(Wih_f, Whh_f, bih_f, bhh_f), (Wih_b, Whh_b, bih_b, bhh_b)]):
        for nm, W in (("ih", Wih), ("hh", Whh)):
            WT = np.ascontiguousarray(W.T).astype(BF)      # (I, 3H)
            wb[f"w{nm}{d}"] = np.ascontiguousarray(WT.reshape(2, 128, 3 * H))
    bias = np.zeros((1, 2048), dtype=BF)
    bnhh = np.zeros((128, 4), dtype=np.float32)
    for d, (bih, bhh) in enumerate([(bih_f, bhh_f), (bih_b, bhh_b)]):
        bias[0, 1024 * d:1024 * d + 512] = (bih[:512] + bhh[:512]).astype(BF)
        bias[0, 1024 * d + 512:1024 * d + 768] = bih[512:].astype(BF)
        bias[0, 1024 * d + 768:1024 * d + 1024] = bhh[512:].astype(BF)
        bnhh[:, 2 * d] = bhh[512:640]
        bnhh[:, 2 * d + 1] = bhh[640:768]
    wb["biasbf"] = bias
    wb["bnhh32"] = bnhh
    wb["identbf"] = np.eye(128, dtype=BF)
    wb["onesbf"] = np.ones((1, CHUNK), dtype=BF)
    return wb


def make_in_maps(p, X, wb):
    BL = p.BL
    in_maps = []
    for c in range(p.ncores):
        Xc = np.ascontiguousarray(
            X[:, c * BL:(c + 1) * BL, :]).reshape(p.T * BL, I).astype(BF)
        m = {"xb": Xc, "gidx": p.gidx[c]}
        m.update(wb)
        in_maps.append(m)
    return in_maps


def assemble_output(p, results):
    """Inverse-permute per-core dumps into the full (T, B, 2H) output."""
    T, BL = p.T, p.BL
    out = np.empty((T, p.ncores * BL, 2 * H), dtype=np.float32)
    for c in range(p.ncores):
        for d in range(2):
            hT = np.asarray(results[c]["h" + "fb"[d]])   # (128, 2, PT) f32
            hfull = hT.transpose(1, 0, 2).reshape(2 * 128, p.PT[d])
            c2r = p.col2row[c][d]
            valid = c2r >= 0
            block = hfull[:, valid].T                    # (nvalid, 256)
            rows = c2r[valid]
            t_idx = rows // BL
            b_idx = rows % BL
            out[t_idx, c * BL + b_idx, d * H:(d + 1) * H] = block
    return out


def kernel(**inputs):
    X = np.asarray(inputs["X"], dtype=np.float32)
    D = np.asarray(inputs["D"])
    p = make_plan(D)
    wb = _shared_consts(*[np.asarray(inputs[k], dtype=np.float32) for k in
                          ("Wih_f", "Whh_f", "bih_f", "bhh_f",
                           "Wih_b", "Whh_b", "bih_b", "bhh_b")])
    nc = build_program(p)
    in_maps = make_in_maps(p, X, wb)
    res = run_bass_kernel_spmd(nc, in_maps, list(range(p.ncores)))
    out = assemble_output(p, res.results)
    return out


# revision 12
# speedup vs baseline: 29.4337x; 29.4337x over previous
"""Trainium2 Bass kernel for nn_DiscontinuedGRU (bidirectional masked GRU).

Math (per direction, torch GRUCell):
    gx = x @ Wih.T + bih ; gh = h @ Whh.T + bhh
    r = sigmoid(gxr+ghr); z = sigmoid(gxz+ghz); n = tanh(gxn + r*ghn)
    h' = (1-z)*n + z*h_in,  h_in = 0 whenever the mask resets (else prev h)

Key idea: the Bernoulli(0.5) reset mask D chops every (batch, direction)
sequence into independent segments (mean length 2, max ~20).  Host code
derives the segments from D, sorts them by length descending, and the
device processes "round k" = the k-th element of every segment as one large
batched GRU-cell evaluation (big matmuls + gates).  Sequential depth
collapses from 2048 steps to ~20 rounds.  Batch is sharded 8 ways (8
batch/core); each core runs both directions for its slice.

Device layout: feature-on-partition (H=256 -> 2 chunks of 128).  x rows are
gathered+transposed by dma_gather(transpose=True) directly into matmul rhs
layout.  Round k's h lives in an SBUF buffer (A/B alternating per round);
each finished round is dumped densely (bf16, feature-major, round-compacted
column order) and the host inverse-permutes into the final (SEQ,B,2H)
output.
"""

import os
import sys
from contextlib import ExitStack

for _p in ("/opt/trn_rl_repo", "/root/.axon_site/_ro/trn_rl_repo"):
    if os.path.isdir(_p) and _p not in sys.path:
        sys.path.insert(0, _p)

import numpy as np
import ml_dtypes

import concourse.bass as bass
import concourse.tile as tile
from concourse import bacc, mybir
from concourse.bass_utils import run_bass_kernel_spmd

ABL = set(os.environ.get("GRU_ABLATE", "").split(","))

BF = ml_dtypes.bfloat16
F32 = mybir.dt.float32
BF16 = mybir.dt.bfloat16
I16 = mybir.dt.int16

SEQ, B, I, H = 2048, 64, 256, 256
NCORES = 8
CHUNK = 512     # compute chunk (PSUM-bank limited)
GCH = int(os.environ.get("GRU_GCH", "512"))

Sigmoid = mybir.ActivationFunctionType.Sigmoid
Tanh = mybir.ActivationFunctionType.Tanh
MULT = mybir.AluOpType.mult
ADD = mybir.AluOpType.add
SUB = mybir.AluOpType.subtract


# ----------------------------------------------------------------- planning

def _segments(Dloc, reverse):
    """Segments of one direction for one core's (T, BL) mask slice,
    sorted by length descending."""
    T, BL = Dloc.shape
    segs = []
    for b in range(BL):
        if not reverse:
            m = (Dloc[:, b] == 1).copy()
            m[0] = True
            starts = np.flatnonzero(m)
            lens = np.diff(np.append(starts, T))
        else:
            m = np.zeros(T, dtype=bool)
            m[:-1] = Dloc[1:, b] == 1
            m[T - 1] = True
            starts = np.flatnonzero(m)
            lens = np.diff(np.concatenate([[-1], starts]))
        for s, L in zip(starts.tolist(), lens.tolist()):
            segs.append((int(L), int(s), int(b)))
    segs.sort(key=lambda x: -x[0])
    return segs


def _round_up(v, m):
    return (v + m - 1) // m * m


class Plan:
    pass


def make_plan(D, T=SEQ, ncores=NCORES):
    """Global round structure + per-core gather indices & output col maps."""
    BL = D.shape[1] // ncores
    p = Plan()
    p.T, p.BL, p.ncores = T, BL, ncores
    p.core_segs = []
    for c in range(ncores):
        Dloc = np.asarray(D[:, c * BL:(c + 1) * BL])
        p.core_segs.append([_segments(Dloc, False), _segments(Dloc, True)])

    p.K = [0, 0]
    p.Nk = [[], []]
    p.offs = [[], []]
    p.PT = [0, 0]
    for d in range(2):
        K = max(segs[d][0][0] for segs in p.core_segs)
        p.K[d] = K
        off = 0
        for k in range(K):
            n_glob = max(sum(1 for L, _, _ in segs[d] if L > k)
                         for segs in p.core_segs)
            nk = _round_up(max(n_glob, 128), 128)
            if k > 0:
                nk = min(nk, p.Nk[d][k - 1])
            p.Nk[d].append(nk)
            p.offs[d].append(off)
            off += nk
        p.PT[d] = off

    p.gidx = []
    p.col2row = []
    gcols = sum(p.PT) // 16
    p.GCOLS = gcols
    p.gc0 = [0, p.PT[0] // 16]
    for c in range(ncores):
        gi = np.zeros((16, gcols), dtype=np.int16)
        maps = []
        for d in range(2):
            segs = p.core_segs[c][d]
            c2r = np.full(p.PT[d], -1, dtype=np.int32)
            for k in range(p.K[d]):
                nk = p.Nk[d][k]
                nreal = sum(1 for L, _, _ in segs if L > k)
                rows = np.zeros(nk, dtype=np.int64)
                for j in range(nreal):
                    L, s, b = segs[j]
                    t = s + k if d == 0 else s - k
                    rows[j] = t * BL + b
                    c2r[p.offs[d][k] + j] = t * BL + b
                blk = rows.reshape(nk // 16, 16).T.astype(np.int16)
                a0 = p.gc0[d] + p.offs[d][k] // 16
                gi[:, a0:a0 + nk // 16] = blk
            maps.append(c2r)
        p.gidx.append(np.tile(gi, (8, 1)))
        p.col2row.append(maps)
    return p


# ----------------------------------------------------------------- builder

def build_program(p):
    """Emit the SPMD Bass/Tile program for plan p."""
    T, BL = p.T, p.BL
    ROWS = T * BL
    biasmm = "actbias" not in ABL  # default: fold biases via K=1 matmuls
    nc = bacc.Bacc("TRN2", target_bir_lowering=False, debug=False,
                   num_devices=p.ncores)

    X_d = nc.dram_tensor("xb", [ROWS, I], BF16, kind="ExternalInput")
    gidx_d = nc.dram_tensor("gidx", [128, p.GCOLS], I16, kind="ExternalInput")
    w_d = [[nc.dram_tensor(f"w{nm}{d}", [2, 128, 3 * H], BF16,
                           kind="ExternalInput")
            for nm in ("ih", "hh")] for d in range(2)]
    bias_d = nc.dram_tensor("biasbf", [1, 2048], BF16, kind="ExternalInput")
    b32_d = nc.dram_tensor("bias32", [128, 16], F32, kind="ExternalInput")
    ident_d = nc.dram_tensor("identbf", [128, 128], BF16, kind="ExternalInput")
    ones_d = nc.dram_tensor("onesbf", [1, CHUNK], BF16, kind="ExternalInput")
    hout_d = [nc.dram_tensor(f"h{'fb'[d]}", [128, 2, p.PT[d]], BF16,
                             kind="ExternalOutput") for d in range(2)]

    with tile.TileContext(nc) as tc, ExitStack() as ctx:
        cpool = ctx.enter_context(tc.tile_pool(name="consts", bufs=1))
        wt = [[cpool.tile([128, 2, 3 * H], BF16, tag=f"w{i_m}{d}",
                          name=f"w{i_m}{d}")
               for i_m in range(2)] for d in range(2)]
        for d in range(2):
            for i_m in range(2):
                for kk in range(2):
                    nc.sync.dma_start(wt[d][i_m][:, kk, :],
                                      w_d[d][i_m].ap()[kk])
        bias_t = cpool.tile([1, 2048], BF16, tag="biasbf")
        nc.sync.dma_start(bias_t[:], bias_d.ap())
        b32_t = cpool.tile([128, 16], F32, tag="bias32")
        nc.sync.dma_start(b32_t[:], b32_d.ap())
        ident_t = cpool.tile([128, 128], BF16, tag="identbf")
        nc.sync.dma_start(ident_t[:], ident_d.ap())
        ones_t = cpool.tile([1, CHUNK], BF16, tag="onesbf")
        nc.sync.dma_start(ones_t[:], ones_d.ap())
        gidx_t = cpool.tile([128, p.GCOLS], I16, tag="gidx")
        nc.sync.dma_start(gidx_t[:], gidx_d.ap())

        # h buffers: slot A holds even rounds (sized for round 0),
        # slot B odd rounds
        hszA = [p.Nk[d][0] for d in range(2)]
        hszB = [p.Nk[d][1] if p.K[d] > 1 else 128 for d in range(2)]
        hbA = [cpool.tile([128, 2, hszA[d]], BF16, tag=f"hA{d}",
                          name=f"hA{d}") for d in range(2)]
        hbB = [cpool.tile([128, 2, hszB[d]], BF16, tag=f"hB{d}",
                          name=f"hB{d}") for d in range(2)]

        xpool = ctx.enter_context(tc.tile_pool(name="xg", bufs=3))
        rzpool = ctx.enter_context(tc.tile_pool(name="rz", bufs=2))
        t1pool = ctx.enter_context(tc.tile_pool(name="t1", bufs=2))
        npool = ctx.enter_context(tc.tile_pool(name="nt", bufs=2))
        dpool = ctx.enter_context(tc.tile_pool(name="dt", bufs=2))
        epool = ctx.enter_context(tc.tile_pool(name="et", bufs=2))
        prz = ctx.enter_context(tc.tile_pool(name="prz", bufs=1, space="PSUM"))
        pnh = ctx.enter_context(tc.tile_pool(name="pnh", bufs=1, space="PSUM"))
        pni = ctx.enter_context(tc.tile_pool(name="pni", bufs=1, space="PSUM"))

        def emit_chunk(d, k, hprev, hnew, xt, xoff, j0, n):
            """One compute chunk: positions [j0, j0+n) of round k, dir d.
            xt: gather tile whose column 0 is round position xoff."""
            bb = 1024 * d
            P_rz = prz.tile([128, 4, CHUNK], F32, tag="prz", name="P_rz")
            xs = xt[:, :, j0 - xoff:j0 - xoff + n]
            hs = hprev[:, :, j0:j0 + n] if k > 0 else None
            for m in range(4):
                first = True
                if biasmm:
                    nc.tensor.matmul(P_rz[:, m, 0:n],
                                     bias_t[0:1, bb + m * 128:
                                            bb + m * 128 + 128],
                                     ones_t[0:1, 0:n], start=True, stop=False)
                    first = False
                for kk in range(2):
                    nc.tensor.matmul(P_rz[:, m, 0:n],
                                     wt[d][0][:, kk, m * 128:m * 128 + 128],
                                     xs[:, kk, :], start=first,
                                     stop=(k == 0 and kk == 1))
                    first = False
                if k > 0:
                    for kk in range(2):
                        nc.tensor.matmul(P_rz[:, m, 0:n],
                                         wt[d][1][:, kk, m * 128:m * 128 + 128],
                                         hs[:, kk, :],
                                         start=False, stop=(kk == 1))
            P_ni = pni.tile([128, 2, CHUNK], F32, tag="pni", name="P_ni")
            for mm in range(2):
                m = 4 + mm
                first = True
                if biasmm:
                    nc.tensor.matmul(P_ni[:, mm, 0:n],
                                     bias_t[0:1, bb + 512 + mm * 128:
                                            bb + 512 + mm * 128 + 128],
                                     ones_t[0:1, 0:n], start=True, stop=False)
                    first = False
                for kk in range(2):
                    nc.tensor.matmul(P_ni[:, mm, 0:n],
                                     wt[d][0][:, kk, m * 128:m * 128 + 128],
                                     xs[:, kk, :], start=first, stop=False)
                    first = False
            if k > 0:
                P_nh = pnh.tile([128, 2, CHUNK], F32, tag="pnh", name="P_nh")
                for mm in range(2):
                    m = 4 + mm
                    for kk in range(2):
                        nc.tensor.matmul(P_nh[:, mm, 0:n],
                                         wt[d][1][:, kk, m * 128:m * 128 + 128],
                                         hs[:, kk, :],
                                         start=(kk == 0), stop=(kk == 1))
            rz = rzpool.tile([128, 4, n], BF16, tag="rz", name="rz")
            if biasmm:
                nc.scalar.activation(rz[:], P_rz[:, :, 0:n], Sigmoid)
            else:
                for m in range(4):
                    nc.scalar.activation(rz[:, m, :], P_rz[:, m, 0:n],
                                         Sigmoid, bias=b32_t[:, 8 * d + m:
                                                             8 * d + m + 1])
            t1 = t1pool.tile([128, 2, n], BF16, tag="t1", name="t1")
            if k > 0:
                # t1 = (ghn + bhh_n) * r   (bhh_n per-partition)
                for mm in range(2):
                    nc.vector.scalar_tensor_tensor(
                        t1[:, mm, :], P_nh[:, mm, 0:n],
                        b32_t[:, 8 * d + 6 + mm:8 * d + 7 + mm],
                        rz[:, mm, :], ADD, MULT)
            else:
                for mm in range(2):
                    nc.vector.tensor_scalar_mul(
                        t1[:, mm, :], rz[:, mm, :],
                        b32_t[:, 8 * d + 6 + mm:8 * d + 7 + mm])
            for mm in range(2):
                nc.tensor.matmul(P_ni[:, mm, 0:n], ident_t[:],
                                 t1[:, mm, :], start=False, stop=True)
            nt = npool.tile([128, 2, n], BF16, tag="nt", name="nt")
            if biasmm:
                nc.scalar.activation(nt[:], P_ni[:, :, 0:n], Tanh)
            else:
                for mm in range(2):
                    nc.scalar.activation(nt[:, mm, :], P_ni[:, mm, 0:n],
                                         Tanh, bias=b32_t[:, 8 * d + 4 + mm:
                                                          8 * d + 5 + mm])
            hd = hnew[:, :, j0:j0 + n]
            et = epool.tile([128, 2, n], BF16, tag="et", name="et")
            if k > 0:
                dt = dpool.tile([128, 2, n], BF16, tag="dt", name="dt")
                nc.vector.tensor_tensor(dt[:], hs, nt[:], SUB)
                nc.vector.tensor_tensor(et[:], rz[:, 2:4, :], dt[:], MULT)
                nc.vector.tensor_tensor(hd, nt[:], et[:], ADD)
            else:
                nc.vector.tensor_tensor(et[:], rz[:, 2:4, :], nt[:], MULT)
                nc.vector.tensor_tensor(hd, nt[:], et[:], SUB)

        for _rep in range(int(os.environ.get("GRU_REPEAT", "1"))):
            maxK = max(p.K)
            for k in range(maxK):
                for d in range(2):
                    if k >= p.K[d]:
                        continue
                    hprev = (hbB[d] if k % 2 == 0 else hbA[d]) if k else None
                    hnew = hbA[d] if k % 2 == 0 else hbB[d]
                    nk = p.Nk[d][k]
                    for g0 in range(0, nk, GCH):
                        gn = min(GCH, nk - g0)
                        xt = xpool.tile([128, 2, gn], BF16, tag="xg",
                                        name="xt")
                        a0 = p.gc0[d] + (p.offs[d][k] + g0) // 16
                        nc.gpsimd.dma_gather(
                            out_ap=xt[:], in_ap=X_d.ap(),
                            idxs_ap=gidx_t[:, a0:a0 + gn // 16],
                            num_idxs=gn, num_idxs_reg=gn, elem_size=I,
                            transpose=True)
                        for j0 in range(g0, g0 + gn, CHUNK):
                            emit_chunk(d, k, hprev, hnew, xt, g0, j0,
                                       min(CHUNK, g0 + gn - j0))
                    c0 = p.offs[d][k]
                    nc.sync.dma_start(hout_d[d].ap()[:, :, c0:c0 + nk],
                                      hnew[:, :, 0:nk])

    nc.compile()
    return nc


# ------------------------------------------------------------- host driver

def _shared_consts(Wih_f, Whh_f, bih_f, bhh_f, Wih_b, Whh_b, bih_b, bhh_b):
    wb = {}
    for d, (Wih, Whh) in enumerate([(Wih_f, Whh_f), (Wih_b, Whh_b)]):
        for nm, W in (("ih", Wih), ("hh", Whh)):
            WT = np.ascontiguousarray(W.T).astype(BF)      # (I, 3H)
            wb[f"w{nm}{d}"] = np.ascontiguousarray(WT.reshape(2, 128, 3 * H))
    bias = np.zeros((1, 2048), dtype=BF)
    b32 = np.zeros((128, 16), dtype=np.float32)
    for d, (bih, bhh) in enumerate([(bih_f, bhh_f), (bih_b, bhh_b)]):
        bias[0, 1024 * d:1024 * d + 512] = (bih[:512] + bhh[:512]).astype(BF)
        bias[0, 1024 * d + 512:1024 * d + 768] = bih[512:].astype(BF)
        brz = (bih[:512] + bhh[:512]).astype(np.float32)
        for m in range(4):
            b32[:, 8 * d + m] = brz[m * 128:(m + 1) * 128]
        b32[:, 8 * d + 4] = bih[512:640]
        b32[:, 8 * d + 5] = bih[640:768]
        b32[:, 8 * d + 6] = bhh[512:640]
        b32[:, 8 * d + 7] = bhh[640:768]
    wb["biasbf"] = bias
    wb["bias32"] = b32
    wb["identbf"] = np.eye(128, dtype=BF)
    wb["onesbf"] = np.ones((1, CHUNK), dtype=BF)
    return wb


def make_in_maps(p, X, wb):
    BL = p.BL
    in_maps = []
    for c in range(p.ncores):
        Xc = np.ascontiguousarray(
            X[:, c * BL:(c + 1) * BL, :]).reshape(p.T * BL, I).astype(BF)
        m = {"xb": Xc, "gidx": p.gidx[c]}
        m.update(wb)
        in_maps.append(m)
    return in_maps


def assemble_output(p, results):
    """Inverse-permute per-core dumps into the full (T, B, 2H) output."""
    T, BL = p.T, p.BL
    out = np.empty((T, p.ncores * BL, 2 * H), dtype=np.float32)
    for c in range(p.ncores):
        for d in range(2):
            hT = np.asarray(results[c]["h" + "fb"[d]]).astype(np.float32)
            hfull = hT.transpose(1, 0, 2).reshape(2 * 128, p.PT[d])
            c2r = p.col2row[c][d]
            valid = c2r >= 0
            block = hfull[:, valid].T
            rows = c2r[valid]
            out[rows // BL, c * BL + rows % BL, d * H:(d + 1) * H] = block
    return out


def kernel(**inputs):
    X = np.asarray(inputs["X"], dtype=np.float32)
    D = np.asarray(inputs["D"])
    p = make_plan(D)
    wb = _shared_consts(*[np.asarray(inputs[k], dtype=np.float32) for k in
                          ("Wih_f", "Whh_f", "bih_f", "bhh_f",
                           "Wih_b", "Whh_b", "bih_b", "bhh_b")])
    nc = build_program(p)
    in_maps = make_in_maps(p, X, wb)
    res = run_bass_kernel_spmd(nc, in_maps, list(range(p.ncores)))
    return assemble_output(p, res.results)


# revision 14
# speedup vs baseline: 31.2362x; 1.0612x over previous
"""Trainium2 Bass kernel for nn_DiscontinuedGRU (bidirectional masked GRU).

Math (per direction, torch GRUCell):
    gx = x @ Wih.T + bih ; gh = h @ Whh.T + bhh
    r = sigmoid(gxr+ghr); z = sigmoid(gxz+ghz); n = tanh(gxn + r*ghn)
    h' = (1-z)*n + z*h_in,  h_in = 0 whenever the mask resets (else prev h)

Key idea: the Bernoulli(0.5) reset mask D chops every (batch, direction)
sequence into independent segments (mean length 2, max ~20).  Host code
derives the segments from D, sorts them by length descending, and the
device processes "round k" = the k-th element of every segment as one large
batched GRU-cell evaluation (big matmuls + gates).  Sequential depth
collapses from 2048 steps to ~20 rounds.  Batch is sharded 8 ways (8
batch/core); each core runs both directions for its slice.

Device layout: feature-on-partition (H=256 -> 2 chunks of 128).  x rows are
gathered+transposed by dma_gather(transpose=True) directly into matmul rhs
layout.  Round k's h lives in an SBUF buffer (A/B alternating per round);
each finished round is dumped densely (bf16, feature-major, round-compacted
column order) and the host inverse-permutes into the final (SEQ,B,2H)
output.
"""

import os
import sys
from contextlib import ExitStack

for _p in ("/opt/trn_rl_repo", "/root/.axon_site/_ro/trn_rl_repo"):
    if os.path.isdir(_p) and _p not in sys.path:
        sys.path.insert(0, _p)

import numpy as np
import ml_dtypes

import concourse.bass as bass
import concourse.tile as tile
from concourse import bacc, mybir
from concourse.bass_utils import run_bass_kernel_spmd

ABL = set(os.environ.get("GRU_ABLATE", "").split(","))

BF = ml_dtypes.bfloat16
F32 = mybir.dt.float32
BF16 = mybir.dt.bfloat16
I16 = mybir.dt.int16

SEQ, B, I, H = 2048, 64, 256, 256
NCORES = 8
CHUNK = 512     # compute chunk (PSUM-bank limited)
GCH = int(os.environ.get("GRU_GCH", "2048"))

Sigmoid = mybir.ActivationFunctionType.Sigmoid
Tanh = mybir.ActivationFunctionType.Tanh
MULT = mybir.AluOpType.mult
ADD = mybir.AluOpType.add
SUB = mybir.AluOpType.subtract


# ----------------------------------------------------------------- planning

def _segments(Dloc, reverse):
    """Segments of one direction for one core's (T, BL) mask slice,
    sorted by length descending."""
    T, BL = Dloc.shape
    segs = []
    for b in range(BL):
        if not reverse:
            m = (Dloc[:, b] == 1).copy()
            m[0] = True
            starts = np.flatnonzero(m)
            lens = np.diff(np.append(starts, T))
        else:
            m = np.zeros(T, dtype=bool)
            m[:-1] = Dloc[1:, b] == 1
            m[T - 1] = True
            starts = np.flatnonzero(m)
            lens = np.diff(np.concatenate([[-1], starts]))
        for s, L in zip(starts.tolist(), lens.tolist()):
            segs.append((int(L), int(s), int(b)))
    segs.sort(key=lambda x: -x[0])
    return segs


def _round_up(v, m):
    return (v + m - 1) // m * m


class Plan:
    pass


def make_plan(D, T=SEQ, ncores=NCORES):
    """Global round structure + per-core gather indices & output col maps."""
    BL = D.shape[1] // ncores
    p = Plan()
    p.T, p.BL, p.ncores = T, BL, ncores
    p.core_segs = []
    for c in range(ncores):
        Dloc = np.asarray(D[:, c * BL:(c + 1) * BL])
        p.core_segs.append([_segments(Dloc, False), _segments(Dloc, True)])

    p.K = [0, 0]
    p.Nk = [[], []]
    p.offs = [[], []]
    p.PT = [0, 0]
    for d in range(2):
        K = max(segs[d][0][0] for segs in p.core_segs)
        p.K[d] = K
        off = 0
        for k in range(K):
            n_glob = max(sum(1 for L, _, _ in segs[d] if L > k)
                         for segs in p.core_segs)
            nk = _round_up(max(n_glob, 128), 128)
            if k > 0:
                nk = min(nk, p.Nk[d][k - 1])
            p.Nk[d].append(nk)
            p.offs[d].append(off)
            off += nk
        p.PT[d] = off

    p.xrows = []    # [core] -> (PT0+PT1,) int64 permuted X-row list
    p.col2row = []  # [core][dir] -> (PT,) int32, -1 = padding
    p.xc0 = [0, p.PT[0]]
    for c in range(ncores):
        rows_all = np.zeros(p.PT[0] + p.PT[1], dtype=np.int64)
        maps = []
        for d in range(2):
            segs = p.core_segs[c][d]
            c2r = np.full(p.PT[d], -1, dtype=np.int32)
            for k in range(p.K[d]):
                nreal = sum(1 for L, _, _ in segs if L > k)
                for j in range(nreal):
                    L, s, b = segs[j]
                    t = s + k if d == 0 else s - k
                    rows_all[p.xc0[d] + p.offs[d][k] + j] = t * BL + b
                    c2r[p.offs[d][k] + j] = t * BL + b
            maps.append(c2r)
        p.xrows.append(rows_all)
        p.col2row.append(maps)
    return p


# ----------------------------------------------------------------- builder

def build_program(p):
    """Emit the SPMD Bass/Tile program for plan p."""
    T, BL = p.T, p.BL
    ROWS = T * BL
    biasmm = "actbias" not in ABL  # default: fold biases via K=1 matmuls
    nc = bacc.Bacc("TRN2", target_bir_lowering=False, debug=False,
                   num_devices=p.ncores)

    PTsum = p.PT[0] + p.PT[1]
    X_d = nc.dram_tensor("xg", [128, 2, PTsum], BF16, kind="ExternalInput")
    w_d = [[nc.dram_tensor(f"w{nm}{d}", [2, 128, 3 * H], BF16,
                           kind="ExternalInput")
            for nm in ("ih", "hh")] for d in range(2)]
    bias_d = nc.dram_tensor("biasbf", [1, 2048], BF16, kind="ExternalInput")
    b32_d = nc.dram_tensor("bias32", [128, 16], F32, kind="ExternalInput")
    ident_d = nc.dram_tensor("identbf", [128, 128], BF16, kind="ExternalInput")
    ones_d = nc.dram_tensor("onesbf", [1, CHUNK], BF16, kind="ExternalInput")
    hout_d = [nc.dram_tensor(f"h{'fb'[d]}", [128, 2, p.PT[d]], BF16,
                             kind="ExternalOutput") for d in range(2)]

    with tile.TileContext(nc) as tc, ExitStack() as ctx:
        cpool = ctx.enter_context(tc.tile_pool(name="consts", bufs=1))
        wt = [[cpool.tile([128, 2, 3 * H], BF16, tag=f"w{i_m}{d}",
                          name=f"w{i_m}{d}")
               for i_m in range(2)] for d in range(2)]
        for d in range(2):
            for i_m in range(2):
                for kk in range(2):
                    nc.sync.dma_start(wt[d][i_m][:, kk, :],
                                      w_d[d][i_m].ap()[kk])
        bias_t = cpool.tile([1, 2048], BF16, tag="biasbf")
        nc.sync.dma_start(bias_t[:], bias_d.ap())
        b32_t = cpool.tile([128, 16], F32, tag="bias32")
        nc.sync.dma_start(b32_t[:], b32_d.ap())
        ident_t = cpool.tile([128, 128], BF16, tag="identbf")
        nc.sync.dma_start(ident_t[:], ident_d.ap())
        ones_t = cpool.tile([1, CHUNK], BF16, tag="onesbf")
        nc.sync.dma_start(ones_t[:], ones_d.ap())

        # h buffers: slot A holds even rounds (sized for round 0),
        # slot B odd rounds
        hszA = [p.Nk[d][0] for d in range(2)]
        hszB = [p.Nk[d][1] if p.K[d] > 1 else 128 for d in range(2)]
        hbA = [cpool.tile([128, 2, hszA[d]], BF16, tag=f"hA{d}",
                          name=f"hA{d}") for d in range(2)]
        hbB = [cpool.tile([128, 2, hszB[d]], BF16, tag=f"hB{d}",
                          name=f"hB{d}") for d in range(2)]

        xpool = ctx.enter_context(tc.tile_pool(name="xg", bufs=3))
        rzpool = ctx.enter_context(tc.tile_pool(name="rz", bufs=2))
        t1pool = ctx.enter_context(tc.tile_pool(name="t1", bufs=2))
        npool = ctx.enter_context(tc.tile_pool(name="nt", bufs=2))
        dpool = ctx.enter_context(tc.tile_pool(name="dt", bufs=2))
        epool = ctx.enter_context(tc.tile_pool(name="et", bufs=2))
        prz = ctx.enter_context(tc.tile_pool(name="prz", bufs=1, space="PSUM"))
        pnh = ctx.enter_context(tc.tile_pool(name="pnh", bufs=1, space="PSUM"))
        pni = ctx.enter_context(tc.tile_pool(name="pni", bufs=1, space="PSUM"))

        def emit_chunk(d, k, hprev, hnew, xt, xoff, j0, n):
            """One compute chunk: positions [j0, j0+n) of round k, dir d.
            xt: gather tile whose column 0 is round position xoff."""
            bb = 1024 * d
            P_rz = prz.tile([128, 4, CHUNK], F32, tag="prz", name="P_rz")
            xs = xt[:, :, j0 - xoff:j0 - xoff + n]
            hs = hprev[:, :, j0:j0 + n] if k > 0 else None
            for m in range(4):
                first = True
                if biasmm:
                    nc.tensor.matmul(P_rz[:, m, 0:n],
                                     bias_t[0:1, bb + m * 128:
                                            bb + m * 128 + 128],
                                     ones_t[0:1, 0:n], start=True, stop=False)
                    first = False
                for kk in range(2):
                    nc.tensor.matmul(P_rz[:, m, 0:n],
                                     wt[d][0][:, kk, m * 128:m * 128 + 128],
                                     xs[:, kk, :], start=first,
                                     stop=(k == 0 and kk == 1))
                    first = False
                if k > 0:
                    for kk in range(2):
                        nc.tensor.matmul(P_rz[:, m, 0:n],
                                         wt[d][1][:, kk, m * 128:m * 128 + 128],
                                         hs[:, kk, :],
                                         start=False, stop=(kk == 1))
            P_ni = pni.tile([128, 2, CHUNK], F32, tag="pni", name="P_ni")
            for mm in range(2):
                m = 4 + mm
                first = True
                if biasmm:
                    nc.tensor.matmul(P_ni[:, mm, 0:n],
                                     bias_t[0:1, bb + 512 + mm * 128:
                                            bb + 512 + mm * 128 + 128],
                                     ones_t[0:1, 0:n], start=True, stop=False)
                    first = False
                for kk in range(2):
                    nc.tensor.matmul(P_ni[:, mm, 0:n],
                                     wt[d][0][:, kk, m * 128:m * 128 + 128],
                                     xs[:, kk, :], start=first, stop=False)
                    first = False
            if k > 0:
                P_nh = pnh.tile([128, 2, CHUNK], F32, tag="pnh", name="P_nh")
                for mm in range(2):
                    m = 4 + mm
                    for kk in range(2):
                        nc.tensor.matmul(P_nh[:, mm, 0:n],
                                         wt[d][1][:, kk, m * 128:m * 128 + 128],
                                         hs[:, kk, :],
                                         start=(kk == 0), stop=(kk == 1))
            rz = rzpool.tile([128, 4, n], BF16, tag="rz", name="rz")
            if biasmm:
                nc.scalar.activation(rz[:], P_rz[:, :, 0:n], Sigmoid)
            else:
                for m in range(4):
                    nc.scalar.activation(rz[:, m, :], P_rz[:, m, 0:n],
                                         Sigmoid, bias=b32_t[:, 8 * d + m:
                                                             8 * d + m + 1])
            t1 = t1pool.tile([128, 2, n], BF16, tag="t1", name="t1")
            if k > 0:
                # t1 = (ghn + bhh_n) * r   (bhh_n per-partition)
                for mm in range(2):
                    nc.vector.scalar_tensor_tensor(
                        t1[:, mm, :], P_nh[:, mm, 0:n],
                        b32_t[:, 8 * d + 6 + mm:8 * d + 7 + mm],
                        rz[:, mm, :], ADD, MULT)
            else:
                for mm in range(2):
                    nc.vector.tensor_scalar_mul(
                        t1[:, mm, :], rz[:, mm, :],
                        b32_t[:, 8 * d + 6 + mm:8 * d + 7 + mm])
            for mm in range(2):
                nc.tensor.matmul(P_ni[:, mm, 0:n], ident_t[:],
                                 t1[:, mm, :], start=False, stop=True)
            nt = npool.tile([128, 2, n], BF16, tag="nt", name="nt")
            if biasmm:
                nc.scalar.activation(nt[:], P_ni[:, :, 0:n], Tanh)
            else:
                for mm in range(2):
                    nc.scalar.activation(nt[:, mm, :], P_ni[:, mm, 0:n],
                                         Tanh, bias=b32_t[:, 8 * d + 4 + mm:
                                                          8 * d + 5 + mm])
            hd = hnew[:, :, j0:j0 + n]
            et = epool.tile([128, 2, n], BF16, tag="et", name="et")
            if k > 0:
                dt = dpool.tile([128, 2, n], BF16, tag="dt", name="dt")
                nc.vector.tensor_tensor(dt[:], hs, nt[:], SUB)
                nc.vector.tensor_tensor(et[:], rz[:, 2:4, :], dt[:], MULT)
                nc.vector.tensor_tensor(hd, nt[:], et[:], ADD)
            else:
                nc.vector.tensor_tensor(et[:], rz[:, 2:4, :], nt[:], MULT)
                nc.vector.tensor_tensor(hd, nt[:], et[:], SUB)

        for _rep in range(int(os.environ.get("GRU_REPEAT", "1"))):
            maxK = max(p.K)
            for k in range(maxK):
                for d in range(2):
                    if k >= p.K[d]:
                        continue
                    hprev = (hbB[d] if k % 2 == 0 else hbA[d]) if k else None
                    hnew = hbA[d] if k % 2 == 0 else hbB[d]
                    nk = p.Nk[d][k]
                    for g0 in range(0, nk, GCH):
                        gn = min(GCH, nk - g0)
                        xt = xpool.tile([128, 2, gn], BF16, tag="xg",
                                        name="xt")
                        a0 = p.xc0[d] + p.offs[d][k] + g0
                        nc.sync.dma_start(xt[:], X_d.ap()[:, :, a0:a0 + gn])
                        for j0 in range(g0, g0 + gn, CHUNK):
                            emit_chunk(d, k, hprev, hnew, xt, g0, j0,
                                       min(CHUNK, g0 + gn - j0))
                    c0 = p.offs[d][k]
                    nc.scalar.dma_start(hout_d[d].ap()[:, :, c0:c0 + nk],
                                        hnew[:, :, 0:nk])

    nc.compile()
    return nc


# ------------------------------------------------------------- host driver

def _shared_consts(Wih_f, Whh_f, bih_f, bhh_f, Wih_b, Whh_b, bih_b, bhh_b):
    wb = {}
    for d, (Wih, Whh) in enumerate([(Wih_f, Whh_f), (Wih_b, Whh_b)]):
        for nm, W in (("ih", Wih), ("hh", Whh)):
            WT = np.ascontiguousarray(W.T).astype(BF)      # (I, 3H)
            wb[f"w{nm}{d}"] = np.ascontiguousarray(WT.reshape(2, 128, 3 * H))
    bias = np.zeros((1, 2048), dtype=BF)
    b32 = np.zeros((128, 16), dtype=np.float32)
    for d, (bih, bhh) in enumerate([(bih_f, bhh_f), (bih_b, bhh_b)]):
        bias[0, 1024 * d:1024 * d + 512] = (bih[:512] + bhh[:512]).astype(BF)
        bias[0, 1024 * d + 512:1024 * d + 768] = bih[512:].astype(BF)
        brz = (bih[:512] + bhh[:512]).astype(np.float32)
        for m in range(4):
            b32[:, 8 * d + m] = brz[m * 128:(m + 1) * 128]
        b32[:, 8 * d + 4] = bih[512:640]
        b32[:, 8 * d + 5] = bih[640:768]
        b32[:, 8 * d + 6] = bhh[512:640]
        b32[:, 8 * d + 7] = bhh[640:768]
    wb["biasbf"] = bias
    wb["bias32"] = b32
    wb["identbf"] = np.eye(128, dtype=BF)
    wb["onesbf"] = np.ones((1, CHUNK), dtype=BF)
    return wb


def make_in_maps(p, X, wb):
    BL = p.BL
    in_maps = []
    for c in range(p.ncores):
        Xc = np.ascontiguousarray(
            X[:, c * BL:(c + 1) * BL, :]).reshape(p.T * BL, I).astype(BF)
        R = Xc[p.xrows[c]]                       # (PTsum, 256) permuted
        Xg = np.ascontiguousarray(
            R.reshape(-1, 2, 128).transpose(2, 1, 0))   # (128, 2, PTsum)
        m = {"xg": Xg}
        m.update(wb)
        in_maps.append(m)
    return in_maps


def assemble_output(p, results):
    """Inverse-permute per-core dumps into the full (T, B, 2H) output."""
    T, BL = p.T, p.BL
    out = np.empty((T, p.ncores * BL, 2 * H), dtype=np.float32)
    for c in range(p.ncores):
        for d in range(2):
            hT = np.asarray(results[c]["h" + "fb"[d]]).astype(np.float32)
            hfull = hT.transpose(1, 0, 2).reshape(2 * 128, p.PT[d])
            c2r = p.col2row[c][d]
            valid = c2r >= 0
            block = hfull[:, valid].T
            rows = c2r[valid]
            out[rows // BL, c * BL + rows % BL, d * H:(d + 1) * H] = block
    return out


def kernel(**inputs):
    X = np.asarray(inputs["X"], dtype=np.float32)
    D = np.asarray(inputs["D"])
    p = make_plan(D)
    wb = _shared_consts(*[np.asarray(inputs[k], dtype=np.float32) for k in
                          ("Wih_f", "Whh_f", "bih_f", "bhh_f",
                           "Wih_b", "Whh_b", "bih_b", "bhh_b")])
    nc = build_program(p)
    in_maps = make_in_maps(p, X, wb)
    res = run_bass_kernel_spmd(nc, in_maps, list(range(p.ncores)))
    return assemble_output(p, res.results)
